# revision 57
# baseline (speedup 1.0000x reference)
"""HCR layer (tensor-product Legendre basis -> dense projection) on 8 trn2 cores.

Math: density[b,o] = 1 + sum_f Bfull[b,f] * C[o,f] - C[o,0]
  where Bfull[b, (i,j,k)] = Li(x0)*Lj(x1)*Lk(x2), orthonormal Legendre on [0,1],
  degree 15 -> 16^3 = 4096 features, batch 8192, out 1024.
  Feature 0 of the basis is identically 1, so with C'[:,0] := 1 and
  C'[:,f] := C[:,f] otherwise, density == Bfull @ C'^T exactly — the +1 bias
  and the -C[:,0] correction are both folded into the coefficient matrix.

Sharding: batch 4-way x out 2-way = 8 cores, no communication.
Per core: [2048 batch, 512 out, 4096 feat]. The basis BfullT [feat, batch] is
precomputed host-side in fp16 and streamed tile-wise; the tensor engine runs
512 matmuls (fp16 in, fp32 PSUM accumulate), PE-bound at ~110 us.

Schedule (tuned against neuron-profile traces; the PE is the bottleneck, so
everything else is arranged to keep it busy from ~10 us to the end):
 - ct tiles stream on the ACT HWDGE queue, bf tiles on the SP queue (each
   dma_start costs ~600 ns of issuing-engine time and queues are FIFO; one
   queue can't feed the startup fast enough). bf issue order is strictly
   bf0-first: interleaving bf1 earlier oversubscribes DMA bandwidth exactly
   when batch-half 0 consumes at peak (measured ~10 us of stalls); bf1
   streams during the tail of half 0 and is comfortably resident before
   its first use in half 1.
 - 5 warmup matmuls on a zeroed junk tile run during the initial DMA fill;
   they ramp the PE DVFS p-state (0.65 -> 2.4 GHz over ~3 us) so real
   matmuls start at full clock the moment the first tiles land.
 - batch half 0: kt-major accumulation over all 8 PSUM banks (DMA-friendly:
   needs only tile kt per step). Per-output-row-pair drains (PSUM -> SBUF
   fp16 downcast, split across ACT/DVE) overlap half 1's compute.
 - batch half 1: four ot-major passes (all bf tiles are SBUF-resident by
   then), so each output-row pair completes ~14 us apart and its drain +
   output DMA overlap the next pass. The last pass runs its two 32-matmul
   chains back to back so the first chain's drain overlaps the second
   chain's ~7 us of matmuls; only one 128 KB chunk remains after the final
   matmul.
 - outputs leave as fp16 (halves write traffic; |out| <= ~1k so fp16 adds
   ~2.6e-4 rel err vs a 2e-2 budget); the host upcasts.
"""

from contextlib import ExitStack

import numpy as np

import concourse.bass as bass
import concourse.mybir as mybir
import concourse.tile as tile
from concourse.bass_utils import run_bass_kernel_spmd

M = 15
NDEG = M + 1            # 16
OUT = 1024
BATCH = 8192
NFEAT = NDEG ** 3       # 4096
NB = 4                  # batch shards
NO = 2                  # out shards
BC = BATCH // NB        # 2048 batch per core
OC = OUT // NO          # 512 out per core
KT = NFEAT // 128       # 32 contraction tiles
BH = BC // 2            # 1024: batch half processed per pass
NWARM = 5               # PE p-state warmup matmuls
FP16 = mybir.dt.float16
FP32 = mybir.dt.float32

_cache = {}


class _SplitDrainTileContext(tile.TileContext):
    """TRN2 allows few sem waits per instruction; the default kernel-tail
    drain carries one wait per ticked proc and fails walrus codegen. Split
    the waits across a chain of drains on the sync engine."""

    _MAXW = 1

    def _drain_and_barrier(self, tick_clock, wait_clock):
        from concourse.vector_clock import ScopedClock

        nc = self.nc
        drain0 = nc.sync.drain()
        wait_clock.add_sem_waits(
            drain0.ins, ScopedClock({None: tick_clock.global_clock})
        )
        si = drain0.ins.sync_info
        waits = list(si.on_wait) if si and si.on_wait else []
        if len(waits) > self._MAXW:
            drain0.ins.sync_info = mybir.SyncInfo(
                on_wait=waits[: self._MAXW],
                on_update=list(si.on_update) if si.on_update else [],
            )
            for i in range(self._MAXW, len(waits), self._MAXW):
                d = nc.sync.drain()
                d.ins.sync_info = mybir.SyncInfo(
                    on_wait=waits[i : i + self._MAXW], on_update=[]
                )

        nc.all_engine_barrier()
        assert self.sems is not None
        popped = nc._tile_sem_poison_stack.pop()
        assert popped is self._sem_poison
        nc.clear_and_free_semaphores(list(self.sems.allocated().values()))
        nc.all_engine_barrier()


def _legendre_basis_np(x):
    """Match reference fp32 recurrence exactly. x: [B, D] fp32 -> [B, D, 16]."""
    t = 2.0 * x - 1.0
    ps = [np.ones_like(t), t]
    for k in range(1, M):
        ps.append(((2 * k + 1) * t * ps[k] - k * ps[k - 1]) / (k + 1))
    ps = ps[: M + 1]
    scale = np.sqrt(2.0 * np.arange(M + 1, dtype=x.dtype) + 1.0)
    return np.stack(ps, axis=-1) * scale


def _build_program():
    if "nc" in _cache:
        return _cache["nc"]

    nc = bass.Bass(
        "TRN2", target_bir_lowering=False, debug=False, num_devices=NB * NO
    )

    # BfullT for this core's batch slice, split in two batch halves.
    bf_d = [
        nc.dram_tensor(f"bf{h}", [NFEAT, BH], FP16, kind="ExternalInput").ap()
        for h in range(2)
    ]
    ct_d = nc.dram_tensor("ct", [NFEAT, OC], FP16, kind="ExternalInput").ap()
    out_d = nc.dram_tensor("outT", [OC, BC], FP16, kind="ExternalOutput").ap()

    OTS = [3, 2, 1, 0]  # output-row-pair processing order, everywhere

    with _SplitDrainTileContext(nc) as tc, ExitStack() as ctx:
        ctp = ctx.enter_context(tc.tile_pool(name="ctp", bufs=KT))
        bfp = ctx.enter_context(tc.tile_pool(name="bfp", bufs=2 * KT))
        psp = ctx.enter_context(tc.tile_pool(name="psp", bufs=8, space="PSUM"))
        stp = ctx.enter_context(tc.tile_pool(name="stp", bufs=17))
        msc = ctx.enter_context(tc.tile_pool(name="msc", bufs=2))

        junk = msc.tile([128, 512], FP16, tag="junk", name="junk", bufs=1)
        # fp32 scratch: the 1-elem gpsimd touches then lower to the fast
        # CAST path (~175ns); a same-dtype copy takes a slow DSP COPY.
        scratch = msc.tile([1, 24], FP32, tag="scratch", name="scratch", bufs=1)
        # gpsimd finishes its preamble earliest, so the junk memset (which
        # gates the PE warmup) lands as soon as possible
        nc.gpsimd.memset(junk[:], 0.0)

        # PE DVFS warmup: runs while the first input DMAs are in flight.
        warm = psp.tile([128, 512], FP32, tag="ps", name="warm")
        for _ in range(NWARM):
            nc.tensor.matmul(
                warm[:], lhsT=junk[:, 0:128], rhs=junk[:], start=True, stop=True
            )

        # ct tiles on the ACT HWDGE queue (its own issue bandwidth).
        ct_sb = []
        for kt in range(KT):
            t = ctp.tile([128, OC], FP16, tag="ct", name=f"ct_{kt}")
            nc.scalar.dma_start(out=t[:], in_=ct_d[kt * 128 : (kt + 1) * 128, :])
            ct_sb.append(t)

        # bf tiles on the SP queue, strict bf0-first priority.
        sp_order = [(0, kt) for kt in range(KT)] + [(1, kt) for kt in range(KT)]
        bf_sb = [[None] * KT, [None] * KT]
        for h, kt in sp_order:
            t = bfp.tile([128, BH], FP16, tag="bf", name=f"bf_{h}_{kt}")
            if h == 0 and kt == 0:
                # Split the very first tile in column halves: the first
                # matmul reads only cols 0:512, so it starts half a
                # transfer (~0.4us) earlier; warmups are tuned to end then.
                nc.sync.dma_start(out=t[:, 0:512], in_=bf_d[0][0:128, 0:512])
                nc.sync.dma_start(out=t[:, 512:BH], in_=bf_d[0][0:128, 512:BH])
            else:
                nc.sync.dma_start(
                    out=t[:], in_=bf_d[h][kt * 128 : (kt + 1) * 128, :]
                )
            bf_sb[h][kt] = t

        def drain_pair(ps_pair, ot, h, tag):
            """PSUM pair -> SBUF fp16 (ACT+DVE in parallel) -> DRAM.
            The 1-elem gpsimd reads absorb the copy-engine waits onto the
            gpsimd stream, so each DMA carries only its queue sem."""
            g0 = len(drained)
            for b2 in range(2):
                st = stp.tile([128, 512], FP16, tag="st", name=f"st_{tag}_{b2}")
                if b2 == 0:
                    nc.scalar.copy(st[:], ps_pair[0][:])
                else:
                    nc.vector.tensor_copy(st[:], ps_pair[1][:])
                g = g0 + b2
                nc.gpsimd.tensor_copy(scratch[:, g : g + 1], st[0:1, 0:1])
                nc.gpsimd.dma_start(
                    out=out_d[
                        ot * 128 : (ot + 1) * 128,
                        h * BH + b2 * 512 : h * BH + (b2 + 1) * 512,
                    ],
                    in_=st[:],
                )
                drained.append(None)

        drained = []

        # ---- batch half 0: kt-major over all 8 PSUM banks ----
        ps0 = {}
        for ot in OTS:
            for b2 in range(2):
                ps0[(ot, b2)] = psp.tile(
                    [128, 512], FP32, tag="ps", name=f"ps0_{ot}_{b2}"
                )
        for s in range(KT):
            # Dummy weight load touching the ct tile: absorbs the ACT-queue
            # DMA wait so the first matmul carries only the SP-queue wait
            # (TRN2 allows one sem wait per instruction).
            nc.tensor.ldweights(ct_sb[s][:, 0:128])
            for ot in OTS:
                lhsT = ct_sb[s][:, ot * 128 : (ot + 1) * 128]
                for b2 in range(2):
                    nc.tensor.matmul(
                        ps0[(ot, b2)][:],
                        lhsT=lhsT,
                        rhs=bf_sb[0][s][:, b2 * 512 : (b2 + 1) * 512],
                        start=(s == 0),
                        stop=(s == KT - 1),
                    )
        for ot in OTS:
            drain_pair((ps0[(ot, 0)], ps0[(ot, 1)]), ot, 0, f"h0_{ot}")

        # ---- batch half 1: four ot-major passes, drains overlap compute ----
        for ot in OTS:
            pair = [
                psp.tile([128, 512], FP32, tag="ps", name=f"ps1_{ot}_{b2}")
                for b2 in range(2)
            ]
            if ot == OTS[0]:
                # absorb the SP-queue wait for bf1[0] so the first matmul
                # carries only the PSUM-free (ACT copy) wait
                nc.tensor.ldweights(bf_sb[1][0][:, 0:128])
            if ot != OTS[-1]:
                for kt in range(KT):
                    lhsT = ct_sb[kt][:, ot * 128 : (ot + 1) * 128]
                    for b2 in range(2):
                        nc.tensor.matmul(
                            pair[b2][:],
                            lhsT=lhsT,
                            rhs=bf_sb[1][kt][:, b2 * 512 : (b2 + 1) * 512],
                            start=(kt == 0),
                            stop=(kt == KT - 1),
                        )
                drain_pair(pair, ot, 1, f"h1_{ot}")
            else:
                # Last pass: run the two 32-matmul chains back to back so the
                # first chain's drain + output DMA overlap the second chain's
                # ~7us of matmuls; only one 128KB chunk remains after the
                # final matmul.
                for b2 in range(2):
                    for kt in range(KT):
                        nc.tensor.matmul(
                            pair[b2][:],
                            lhsT=ct_sb[kt][:, ot * 128 : (ot + 1) * 128],
                            rhs=bf_sb[1][kt][:, b2 * 512 : (b2 + 1) * 512],
                            start=(kt == 0),
                            stop=(kt == KT - 1),
                        )
                    c0 = BH + b2 * 512
                    if b2 == 0:
                        st = stp.tile(
                            [128, 512], FP16, tag="st", name=f"st_h1_{ot}_{b2}"
                        )
                        nc.scalar.copy(st[:], pair[0][:])
                        g = len(drained)
                        nc.gpsimd.tensor_copy(scratch[:, g : g + 1], st[0:1, 0:1])
                        nc.gpsimd.dma_start(
                            out=out_d[ot * 128 : (ot + 1) * 128, c0 : c0 + 512],
                            in_=st[:],
                        )
                    else:
                        # Very last chunk: ACT and DVE copy one half each
                        # into separate staging tiles in parallel, halving
                        # the copy latency on the exposed tail path.
                        g = len(drained)
                        sthalves = []
                        for half, ecopy in ((0, nc.scalar.copy),
                                            (1, nc.vector.tensor_copy)):
                            sth = stp.tile(
                                [128, 256], FP16, tag="st",
                                name=f"st_h1_{ot}_{b2}_{half}",
                            )
                            if half == 1:
                                # The framework serializes the two PSUM-bank
                                # readers; a 1-elem DVE touch absorbs the
                                # ACT-copy dep so the real copy carries only
                                # the PE chain-stop wait.
                                nc.vector.tensor_copy(
                                    scratch[:, 20:21], sthalves[0][0:1, 0:1]
                                )
                            sthalves.append(sth)
                            ecopy(
                                sth[:],
                                pair[1][:, half * 256 : (half + 1) * 256],
                            )
                            nc.gpsimd.tensor_copy(
                                scratch[:, g + half : g + half + 1],
                                sth[0:1, 0:1],
                            )
                            nc.gpsimd.dma_start(
                                out=out_d[
                                    ot * 128 : (ot + 1) * 128,
                                    c0 + half * 256 : c0 + (half + 1) * 256,
                                ],
                                in_=sth[:],
                            )
                    drained.append(None)

    _cache["nc"] = nc
    return nc


def _make_in_maps(x, coefficients):
    L = _legendre_basis_np(np.asarray(x, dtype=np.float32))  # [8192, 3, 16]
    CT = np.ascontiguousarray(np.asarray(coefficients, dtype=np.float32).T)
    # Bfull[:, 0] == 1 exactly, so C'[0,:] = 1 yields
    # Bfull @ C'^T == 1 + Bfull @ C^T - C[:,0] (the reference expression).
    CT[0, :] = 1.0
    CT16 = CT.astype(np.float16)

    in_maps = []
    for c in range(NB * NO):
        bs, osh = c % NB, c // NB
        Lb = L[bs * BC : (bs + 1) * BC]  # [BC, 3, 16]
        # BfullT[(i,j,k), b] in fp16, built from fp32 factors
        bfull = np.einsum("bi,bj,bk->ijkb", Lb[:, 0], Lb[:, 1], Lb[:, 2])
        bfull = bfull.reshape(NFEAT, BC).astype(np.float16)
        in_maps.append(
            {
                "bf0": np.ascontiguousarray(bfull[:, :BH]),
                "bf1": np.ascontiguousarray(bfull[:, BH:]),
                "ct": np.ascontiguousarray(CT16[:, osh * OC : (osh + 1) * OC]),
            }
        )
    return in_maps


def _assemble(results):
    out = np.empty((BATCH, OUT), dtype=np.float32)
    for c in range(NB * NO):
        bs, osh = c % NB, c // NB
        out[bs * BC : (bs + 1) * BC, osh * OC : (osh + 1) * OC] = (
            results[c]["outT"].astype(np.float32).T
        )
    return out


def _run(x, coefficients, trace=False, **kwargs):
    nc = _build_program()
    in_maps = _make_in_maps(x, coefficients)
    res = run_bass_kernel_spmd(
        nc, in_maps, list(range(NB * NO)), trace=trace, **kwargs
    )
    return _assemble(res.results), res


def kernel(x, coefficients):
    out, _ = _run(x, coefficients)
    return out


# revision 59
# speedup vs baseline: 1.0020x; 1.0020x over previous
"""HCR layer (tensor-product Legendre basis -> dense projection) on 8 trn2 cores.

Math: density[b,o] = 1 + sum_f Bfull[b,f] * C[o,f] - C[o,0]
  where Bfull[b, (i,j,k)] = Li(x0)*Lj(x1)*Lk(x2), orthonormal Legendre on [0,1],
  degree 15 -> 16^3 = 4096 features, batch 8192, out 1024.
  Feature 0 of the basis is identically 1, so with C'[:,0] := 1 and
  C'[:,f] := C[:,f] otherwise, density == Bfull @ C'^T exactly — the +1 bias
  and the -C[:,0] correction are both folded into the coefficient matrix.

Sharding: batch 4-way x out 2-way = 8 cores, no communication.
Per core: [2048 batch, 512 out, 4096 feat]. The basis BfullT [feat, batch] is
precomputed host-side in fp16 and streamed tile-wise; the tensor engine runs
512 matmuls (fp16 in, fp32 PSUM accumulate), PE-bound at ~110 us.

Schedule (tuned against neuron-profile traces; the PE is the bottleneck, so
everything else is arranged to keep it busy from ~10 us to the end):
 - ct tiles stream on the ACT HWDGE queue, bf tiles on the SP queue (each
   dma_start costs ~600 ns of issuing-engine time and queues are FIFO; one
   queue can't feed the startup fast enough). bf issue order is strictly
   bf0-first: interleaving bf1 earlier oversubscribes DMA bandwidth exactly
   when batch-half 0 consumes at peak (measured ~10 us of stalls); bf1
   streams during the tail of half 0 and is comfortably resident before
   its first use in half 1.
 - 5 warmup matmuls on a zeroed junk tile run during the initial DMA fill;
   they ramp the PE DVFS p-state (0.65 -> 2.4 GHz over ~3 us) so real
   matmuls start at full clock the moment the first tiles land.
 - batch half 0: kt-major accumulation over all 8 PSUM banks (DMA-friendly:
   needs only tile kt per step). Per-output-row-pair drains (PSUM -> SBUF
   fp16 downcast, split across ACT/DVE) overlap half 1's compute.
 - batch half 1: four ot-major passes (all bf tiles are SBUF-resident by
   then), so each output-row pair completes ~14 us apart and its drain +
   output DMA overlap the next pass. The last pass runs its two 32-matmul
   chains back to back so the first chain's drain overlaps the second
   chain's ~7 us of matmuls; only one 128 KB chunk remains after the final
   matmul.
 - outputs leave as fp16 (halves write traffic; |out| <= ~1k so fp16 adds
   ~2.6e-4 rel err vs a 2e-2 budget); the host upcasts.
"""

from contextlib import ExitStack

import numpy as np

import concourse.bass as bass
import concourse.mybir as mybir
import concourse.tile as tile
from concourse.bass_utils import run_bass_kernel_spmd

M = 15
NDEG = M + 1            # 16
OUT = 1024
BATCH = 8192
NFEAT = NDEG ** 3       # 4096
NB = 4                  # batch shards
NO = 2                  # out shards
BC = BATCH // NB        # 2048 batch per core
OC = OUT // NO          # 512 out per core
KT = NFEAT // 128       # 32 contraction tiles
BH = BC // 2            # 1024: batch half processed per pass
NWARM = 26              # PE p-state warmup matmuls (128-wide)
FP16 = mybir.dt.float16
FP32 = mybir.dt.float32

_cache = {}


class _SplitDrainTileContext(tile.TileContext):
    """TRN2 allows few sem waits per instruction; the default kernel-tail
    drain carries one wait per ticked proc and fails walrus codegen. Split
    the waits across a chain of drains on the sync engine."""

    _MAXW = 1

    def _drain_and_barrier(self, tick_clock, wait_clock):
        from concourse.vector_clock import ScopedClock

        nc = self.nc
        drain0 = nc.sync.drain()
        wait_clock.add_sem_waits(
            drain0.ins, ScopedClock({None: tick_clock.global_clock})
        )
        si = drain0.ins.sync_info
        waits = list(si.on_wait) if si and si.on_wait else []
        if len(waits) > self._MAXW:
            drain0.ins.sync_info = mybir.SyncInfo(
                on_wait=waits[: self._MAXW],
                on_update=list(si.on_update) if si.on_update else [],
            )
            for i in range(self._MAXW, len(waits), self._MAXW):
                d = nc.sync.drain()
                d.ins.sync_info = mybir.SyncInfo(
                    on_wait=waits[i : i + self._MAXW], on_update=[]
                )

        nc.all_engine_barrier()
        assert self.sems is not None
        popped = nc._tile_sem_poison_stack.pop()
        assert popped is self._sem_poison
        nc.clear_and_free_semaphores(list(self.sems.allocated().values()))
        nc.all_engine_barrier()


def _legendre_basis_np(x):
    """Match reference fp32 recurrence exactly. x: [B, D] fp32 -> [B, D, 16]."""
    t = 2.0 * x - 1.0
    ps = [np.ones_like(t), t]
    for k in range(1, M):
        ps.append(((2 * k + 1) * t * ps[k] - k * ps[k - 1]) / (k + 1))
    ps = ps[: M + 1]
    scale = np.sqrt(2.0 * np.arange(M + 1, dtype=x.dtype) + 1.0)
    return np.stack(ps, axis=-1) * scale


def _build_program():
    if "nc" in _cache:
        return _cache["nc"]

    nc = bass.Bass(
        "TRN2", target_bir_lowering=False, debug=False, num_devices=NB * NO
    )

    # BfullT for this core's batch slice, split in two batch halves.
    bf_d = [
        nc.dram_tensor(f"bf{h}", [NFEAT, BH], FP16, kind="ExternalInput").ap()
        for h in range(2)
    ]
    ct_d = nc.dram_tensor("ct", [NFEAT, OC], FP16, kind="ExternalInput").ap()
    out_d = nc.dram_tensor("outT", [OC, BC], FP16, kind="ExternalOutput").ap()

    OTS = [3, 2, 1, 0]  # output-row-pair processing order, everywhere

    with _SplitDrainTileContext(nc) as tc, ExitStack() as ctx:
        ctp = ctx.enter_context(tc.tile_pool(name="ctp", bufs=KT))
        bfp = ctx.enter_context(tc.tile_pool(name="bfp", bufs=2 * KT))
        psp = ctx.enter_context(tc.tile_pool(name="psp", bufs=8, space="PSUM"))
        stp = ctx.enter_context(tc.tile_pool(name="stp", bufs=17))
        msc = ctx.enter_context(tc.tile_pool(name="msc", bufs=2))

        junk = msc.tile([128, 128], FP16, tag="junk", name="junk", bufs=1)
        # fp32 scratch: the 1-elem gpsimd touches then lower to the fast
        # CAST path (~175ns); a same-dtype copy takes a slow DSP COPY.
        scratch = msc.tile([1, 24], FP32, tag="scratch", name="scratch", bufs=1)
        # A minimal 128-col memset (~0.13us on gpsimd) unblocks the
        # warmups right at the PE preamble exit, so the ~3us DVFS ramp
        # completes as the first data tiles land.
        nc.gpsimd.memset(junk[:], 0.0)

        # PE DVFS warmup: runs while the first input DMAs are in flight.
        warm = psp.tile([128, 512], FP32, tag="ps", name="warm")
        for _ in range(NWARM):
            nc.tensor.matmul(
                warm[:, 0:128], lhsT=junk[:], rhs=junk[:],
                start=True, stop=True,
            )

        # ct tiles on the ACT HWDGE queue (its own issue bandwidth).
        ct_sb = []
        for kt in range(KT):
            t = ctp.tile([128, OC], FP16, tag="ct", name=f"ct_{kt}")
            nc.scalar.dma_start(out=t[:], in_=ct_d[kt * 128 : (kt + 1) * 128, :])
            ct_sb.append(t)

        # bf tiles on the SP queue, strict bf0-first priority.
        sp_order = [(0, kt) for kt in range(KT)] + [(1, kt) for kt in range(KT)]
        bf_sb = [[None] * KT, [None] * KT]
        for h, kt in sp_order:
            t = bfp.tile([128, BH], FP16, tag="bf", name=f"bf_{h}_{kt}")
            if h == 0 and kt == 0:
                # Split the very first tile in column halves: the first
                # matmul reads only cols 0:512, so it starts half a
                # transfer (~0.4us) earlier; warmups are tuned to end then.
                nc.sync.dma_start(out=t[:, 0:512], in_=bf_d[0][0:128, 0:512])
                nc.sync.dma_start(out=t[:, 512:BH], in_=bf_d[0][0:128, 512:BH])
            else:
                nc.sync.dma_start(
                    out=t[:], in_=bf_d[h][kt * 128 : (kt + 1) * 128, :]
                )
            bf_sb[h][kt] = t

        def drain_pair(ps_pair, ot, h, tag):
            """PSUM pair -> SBUF fp16 (ACT+DVE in parallel) -> DRAM.
            The 1-elem gpsimd reads absorb the copy-engine waits onto the
            gpsimd stream, so each DMA carries only its queue sem."""
            g0 = len(drained)
            for b2 in range(2):
                st = stp.tile([128, 512], FP16, tag="st", name=f"st_{tag}_{b2}")
                if b2 == 0:
                    nc.scalar.copy(st[:], ps_pair[0][:])
                else:
                    nc.vector.tensor_copy(st[:], ps_pair[1][:])
                g = g0 + b2
                nc.gpsimd.tensor_copy(scratch[:, g : g + 1], st[0:1, 0:1])
                nc.gpsimd.dma_start(
                    out=out_d[
                        ot * 128 : (ot + 1) * 128,
                        h * BH + b2 * 512 : h * BH + (b2 + 1) * 512,
                    ],
                    in_=st[:],
                )
                drained.append(None)

        drained = []

        # ---- batch half 0: kt-major over all 8 PSUM banks ----
        ps0 = {}
        for ot in OTS:
            for b2 in range(2):
                ps0[(ot, b2)] = psp.tile(
                    [128, 512], FP32, tag="ps", name=f"ps0_{ot}_{b2}"
                )
        for s in range(KT):
            # Dummy weight load touching the ct tile: absorbs the ACT-queue
            # DMA wait so the first matmul carries only the SP-queue wait
            # (TRN2 allows one sem wait per instruction).
            nc.tensor.ldweights(ct_sb[s][:, 0:128])
            for ot in OTS:
                lhsT = ct_sb[s][:, ot * 128 : (ot + 1) * 128]
                for b2 in range(2):
                    nc.tensor.matmul(
                        ps0[(ot, b2)][:],
                        lhsT=lhsT,
                        rhs=bf_sb[0][s][:, b2 * 512 : (b2 + 1) * 512],
                        start=(s == 0),
                        stop=(s == KT - 1),
                    )
        for ot in OTS:
            drain_pair((ps0[(ot, 0)], ps0[(ot, 1)]), ot, 0, f"h0_{ot}")

        # ---- batch half 1: four ot-major passes, drains overlap compute ----
        for ot in OTS:
            pair = [
                psp.tile([128, 512], FP32, tag="ps", name=f"ps1_{ot}_{b2}")
                for b2 in range(2)
            ]
            if ot == OTS[0]:
                # absorb the SP-queue wait for bf1[0] so the first matmul
                # carries only the PSUM-free (ACT copy) wait
                nc.tensor.ldweights(bf_sb[1][0][:, 0:128])
            if ot != OTS[-1]:
                for kt in range(KT):
                    lhsT = ct_sb[kt][:, ot * 128 : (ot + 1) * 128]
                    for b2 in range(2):
                        nc.tensor.matmul(
                            pair[b2][:],
                            lhsT=lhsT,
                            rhs=bf_sb[1][kt][:, b2 * 512 : (b2 + 1) * 512],
                            start=(kt == 0),
                            stop=(kt == KT - 1),
                        )
                drain_pair(pair, ot, 1, f"h1_{ot}")
            else:
                # Last pass: run the two 32-matmul chains back to back so the
                # first chain's drain + output DMA overlap the second chain's
                # ~7us of matmuls; only one 128KB chunk remains after the
                # final matmul.
                for b2 in range(2):
                    for kt in range(KT):
                        nc.tensor.matmul(
                            pair[b2][:],
                            lhsT=ct_sb[kt][:, ot * 128 : (ot + 1) * 128],
                            rhs=bf_sb[1][kt][:, b2 * 512 : (b2 + 1) * 512],
                            start=(kt == 0),
                            stop=(kt == KT - 1),
                        )
                    c0 = BH + b2 * 512
                    if b2 == 0:
                        st = stp.tile(
                            [128, 512], FP16, tag="st", name=f"st_h1_{ot}_{b2}"
                        )
                        nc.scalar.copy(st[:], pair[0][:])
                        g = len(drained)
                        nc.gpsimd.tensor_copy(scratch[:, g : g + 1], st[0:1, 0:1])
                        nc.gpsimd.dma_start(
                            out=out_d[ot * 128 : (ot + 1) * 128, c0 : c0 + 512],
                            in_=st[:],
                        )
                    else:
                        # Very last chunk: ACT and DVE copy one half each
                        # into separate staging tiles in parallel, halving
                        # the copy latency on the exposed tail path.
                        g = len(drained)
                        sthalves = []
                        for half, ecopy in ((0, nc.scalar.copy),
                                            (1, nc.vector.tensor_copy)):
                            sth = stp.tile(
                                [128, 256], FP16, tag="st",
                                name=f"st_h1_{ot}_{b2}_{half}",
                            )
                            if half == 1:
                                # The framework serializes the two PSUM-bank
                                # readers; a 1-elem DVE touch absorbs the
                                # ACT-copy dep so the real copy carries only
                                # the PE chain-stop wait.
                                nc.vector.tensor_copy(
                                    scratch[:, 20:21], sthalves[0][0:1, 0:1]
                                )
                            sthalves.append(sth)
                            ecopy(
                                sth[:],
                                pair[1][:, half * 256 : (half + 1) * 256],
                            )
                            nc.gpsimd.tensor_copy(
                                scratch[:, g + half : g + half + 1],
                                sth[0:1, 0:1],
                            )
                            nc.gpsimd.dma_start(
                                out=out_d[
                                    ot * 128 : (ot + 1) * 128,
                                    c0 + half * 256 : c0 + (half + 1) * 256,
                                ],
                                in_=sth[:],
                            )
                    drained.append(None)

    _cache["nc"] = nc
    return nc


def _make_in_maps(x, coefficients):
    L = _legendre_basis_np(np.asarray(x, dtype=np.float32))  # [8192, 3, 16]
    CT = np.ascontiguousarray(np.asarray(coefficients, dtype=np.float32).T)
    # Bfull[:, 0] == 1 exactly, so C'[0,:] = 1 yields
    # Bfull @ C'^T == 1 + Bfull @ C^T - C[:,0] (the reference expression).
    CT[0, :] = 1.0
    CT16 = CT.astype(np.float16)

    in_maps = []
    for c in range(NB * NO):
        bs, osh = c % NB, c // NB
        Lb = L[bs * BC : (bs + 1) * BC]  # [BC, 3, 16]
        # BfullT[(i,j,k), b] in fp16, built from fp32 factors
        bfull = np.einsum("bi,bj,bk->ijkb", Lb[:, 0], Lb[:, 1], Lb[:, 2])
        bfull = bfull.reshape(NFEAT, BC).astype(np.float16)
        in_maps.append(
            {
                "bf0": np.ascontiguousarray(bfull[:, :BH]),
                "bf1": np.ascontiguousarray(bfull[:, BH:]),
                "ct": np.ascontiguousarray(CT16[:, osh * OC : (osh + 1) * OC]),
            }
        )
    return in_maps


def _assemble(results):
    out = np.empty((BATCH, OUT), dtype=np.float32)
    for c in range(NB * NO):
        bs, osh = c % NB, c // NB
        out[bs * BC : (bs + 1) * BC, osh * OC : (osh + 1) * OC] = (
            results[c]["outT"].astype(np.float32).T
        )
    return out


def _run(x, coefficients, trace=False, **kwargs):
    nc = _build_program()
    in_maps = _make_in_maps(x, coefficients)
    res = run_bass_kernel_spmd(
        nc, in_maps, list(range(NB * NO)), trace=trace, **kwargs
    )
    return _assemble(res.results), res


def kernel(x, coefficients):
    out, _ = _run(x, coefficients)
    return out


# revision 60
# speedup vs baseline: 1.0054x; 1.0034x over previous
"""HCR layer (tensor-product Legendre basis -> dense projection) on 8 trn2 cores.

Math: density[b,o] = 1 + sum_f Bfull[b,f] * C[o,f] - C[o,0]
  where Bfull[b, (i,j,k)] = Li(x0)*Lj(x1)*Lk(x2), orthonormal Legendre on [0,1],
  degree 15 -> 16^3 = 4096 features, batch 8192, out 1024.
  Feature 0 of the basis is identically 1, so with C'[:,0] := 1 and
  C'[:,f] := C[:,f] otherwise, density == Bfull @ C'^T exactly — the +1 bias
  and the -C[:,0] correction are both folded into the coefficient matrix.

Sharding: batch 4-way x out 2-way = 8 cores, no communication.
Per core: [2048 batch, 512 out, 4096 feat]. The basis BfullT [feat, batch] is
precomputed host-side in fp16 and streamed tile-wise; the tensor engine runs
512 matmuls (fp16 in, fp32 PSUM accumulate), PE-bound at ~110 us.

Schedule (tuned against neuron-profile traces; the PE is the bottleneck, so
everything else is arranged to keep it busy from ~10 us to the end):
 - ct tiles stream on the ACT HWDGE queue, bf tiles on the SP queue (each
   dma_start costs ~600 ns of issuing-engine time and queues are FIFO; one
   queue can't feed the startup fast enough). bf issue order is strictly
   bf0-first: interleaving bf1 earlier oversubscribes DMA bandwidth exactly
   when batch-half 0 consumes at peak (measured ~10 us of stalls); bf1
   streams during the tail of half 0 and is comfortably resident before
   its first use in half 1.
 - 5 warmup matmuls on a zeroed junk tile run during the initial DMA fill;
   they ramp the PE DVFS p-state (0.65 -> 2.4 GHz over ~3 us) so real
   matmuls start at full clock the moment the first tiles land.
 - batch half 0: kt-major accumulation over all 8 PSUM banks (DMA-friendly:
   needs only tile kt per step). Per-output-row-pair drains (PSUM -> SBUF
   fp16 downcast, split across ACT/DVE) overlap half 1's compute.
 - batch half 1: four ot-major passes (all bf tiles are SBUF-resident by
   then), so each output-row pair completes ~14 us apart and its drain +
   output DMA overlap the next pass. The last pass runs its two 32-matmul
   chains back to back so the first chain's drain overlaps the second
   chain's ~7 us of matmuls; only one 128 KB chunk remains after the final
   matmul.
 - outputs leave as fp16 (halves write traffic; |out| <= ~1k so fp16 adds
   ~2.6e-4 rel err vs a 2e-2 budget); the host upcasts.
"""

from contextlib import ExitStack

import numpy as np

import concourse.bass as bass
import concourse.mybir as mybir
import concourse.tile as tile
from concourse.bass_utils import run_bass_kernel_spmd

M = 15
NDEG = M + 1            # 16
OUT = 1024
BATCH = 8192
NFEAT = NDEG ** 3       # 4096
NB = 4                  # batch shards
NO = 2                  # out shards
BC = BATCH // NB        # 2048 batch per core
OC = OUT // NO          # 512 out per core
KT = NFEAT // 128       # 32 contraction tiles
BH = BC // 2            # 1024: batch half processed per pass
NWARM = 6               # PE p-state warmup matmuls
FP16 = mybir.dt.float16
FP32 = mybir.dt.float32

_cache = {}


class _SplitDrainTileContext(tile.TileContext):
    """TRN2 allows few sem waits per instruction; the default kernel-tail
    drain carries one wait per ticked proc and fails walrus codegen. Split
    the waits across a chain of drains on the sync engine."""

    _MAXW = 1

    def _drain_and_barrier(self, tick_clock, wait_clock):
        from concourse.vector_clock import ScopedClock

        nc = self.nc
        drain0 = nc.sync.drain()
        wait_clock.add_sem_waits(
            drain0.ins, ScopedClock({None: tick_clock.global_clock})
        )
        si = drain0.ins.sync_info
        waits = list(si.on_wait) if si and si.on_wait else []
        if len(waits) > self._MAXW:
            drain0.ins.sync_info = mybir.SyncInfo(
                on_wait=waits[: self._MAXW],
                on_update=list(si.on_update) if si.on_update else [],
            )
            for i in range(self._MAXW, len(waits), self._MAXW):
                d = nc.sync.drain()
                d.ins.sync_info = mybir.SyncInfo(
                    on_wait=waits[i : i + self._MAXW], on_update=[]
                )

        nc.all_engine_barrier()
        assert self.sems is not None
        popped = nc._tile_sem_poison_stack.pop()
        assert popped is self._sem_poison
        nc.clear_and_free_semaphores(list(self.sems.allocated().values()))
        nc.all_engine_barrier()


def _legendre_basis_np(x):
    """Match reference fp32 recurrence exactly. x: [B, D] fp32 -> [B, D, 16]."""
    t = 2.0 * x - 1.0
    ps = [np.ones_like(t), t]
    for k in range(1, M):
        ps.append(((2 * k + 1) * t * ps[k] - k * ps[k - 1]) / (k + 1))
    ps = ps[: M + 1]
    scale = np.sqrt(2.0 * np.arange(M + 1, dtype=x.dtype) + 1.0)
    return np.stack(ps, axis=-1) * scale


def _build_program():
    if "nc" in _cache:
        return _cache["nc"]

    nc = bass.Bass(
        "TRN2", target_bir_lowering=False, debug=False, num_devices=NB * NO
    )

    # BfullT for this core's batch slice, split in two batch halves.
    bf_d = [
        nc.dram_tensor(f"bf{h}", [NFEAT, BH], FP16, kind="ExternalInput").ap()
        for h in range(2)
    ]
    ct_d = nc.dram_tensor("ct", [NFEAT, OC], FP16, kind="ExternalInput").ap()
    out_d = nc.dram_tensor("outT", [OC, BC], FP16, kind="ExternalOutput").ap()

    OTS = [3, 2, 1, 0]  # output-row-pair processing order, everywhere

    with _SplitDrainTileContext(nc) as tc, ExitStack() as ctx:
        ctp = ctx.enter_context(tc.tile_pool(name="ctp", bufs=KT))
        bfp = ctx.enter_context(tc.tile_pool(name="bfp", bufs=2 * KT))
        psp = ctx.enter_context(tc.tile_pool(name="psp", bufs=8, space="PSUM"))
        stp = ctx.enter_context(tc.tile_pool(name="stp", bufs=17))
        msc = ctx.enter_context(tc.tile_pool(name="msc", bufs=2))

        junk = msc.tile([128, 512], FP16, tag="junk", name="junk", bufs=1)
        # fp32 scratch: the 1-elem gpsimd touches then lower to the fast
        # CAST path (~175ns); a same-dtype copy takes a slow DSP COPY.
        scratch = msc.tile([1, 24], FP32, tag="scratch", name="scratch", bufs=1)
        # gpsimd finishes its preamble earliest, so the junk memset (which
        # gates the PE warmup) lands as soon as possible
        nc.gpsimd.memset(junk[:], 0.0)

        # PE DVFS warmup: runs while the first input DMAs are in flight.
        warm = psp.tile([128, 512], FP32, tag="ps", name="warm")
        for _ in range(NWARM):
            nc.tensor.matmul(
                warm[:], lhsT=junk[:, 0:128], rhs=junk[:], start=True, stop=True
            )

        # ct tiles on the ACT HWDGE queue (its own issue bandwidth).
        ct_sb = []
        for kt in range(KT):
            t = ctp.tile([128, OC], FP16, tag="ct", name=f"ct_{kt}")
            nc.scalar.dma_start(out=t[:], in_=ct_d[kt * 128 : (kt + 1) * 128, :])
            ct_sb.append(t)

        # bf tiles on the SP queue, strict bf0-first priority.
        sp_order = [(0, kt) for kt in range(KT)] + [(1, kt) for kt in range(KT)]
        bf_sb = [[None] * KT, [None] * KT]
        for h, kt in sp_order:
            t = bfp.tile([128, BH], FP16, tag="bf", name=f"bf_{h}_{kt}")
            nc.sync.dma_start(out=t[:], in_=bf_d[h][kt * 128 : (kt + 1) * 128, :])
            bf_sb[h][kt] = t

        def drain_pair(ps_pair, ot, h, tag):
            """PSUM pair -> SBUF fp16 (ACT+DVE in parallel) -> DRAM.
            The 1-elem gpsimd reads absorb the copy-engine waits onto the
            gpsimd stream, so each DMA carries only its queue sem."""
            g0 = len(drained)
            for b2 in range(2):
                st = stp.tile([128, 512], FP16, tag="st", name=f"st_{tag}_{b2}")
                if b2 == 0:
                    nc.scalar.copy(st[:], ps_pair[0][:])
                else:
                    nc.vector.tensor_copy(st[:], ps_pair[1][:])
                g = g0 + b2
                nc.gpsimd.tensor_copy(scratch[:, g : g + 1], st[0:1, 0:1])
                nc.gpsimd.dma_start(
                    out=out_d[
                        ot * 128 : (ot + 1) * 128,
                        h * BH + b2 * 512 : h * BH + (b2 + 1) * 512,
                    ],
                    in_=st[:],
                )
                drained.append(None)

        drained = []

        # ---- batch half 0: kt-major over all 8 PSUM banks ----
        ps0 = {}
        for ot in OTS:
            for b2 in range(2):
                ps0[(ot, b2)] = psp.tile(
                    [128, 512], FP32, tag="ps", name=f"ps0_{ot}_{b2}"
                )
        for s in range(KT):
            # Dummy weight load touching the ct tile: absorbs the ACT-queue
            # DMA wait so the first matmul carries only the SP-queue wait
            # (TRN2 allows one sem wait per instruction).
            nc.tensor.ldweights(ct_sb[s][:, 0:128])
            for ot in OTS:
                lhsT = ct_sb[s][:, ot * 128 : (ot + 1) * 128]
                for b2 in range(2):
                    nc.tensor.matmul(
                        ps0[(ot, b2)][:],
                        lhsT=lhsT,
                        rhs=bf_sb[0][s][:, b2 * 512 : (b2 + 1) * 512],
                        start=(s == 0),
                        stop=(s == KT - 1),
                    )
        for ot in OTS:
            drain_pair((ps0[(ot, 0)], ps0[(ot, 1)]), ot, 0, f"h0_{ot}")

        # ---- batch half 1: four ot-major passes, drains overlap compute ----
        for ot in OTS:
            pair = [
                psp.tile([128, 512], FP32, tag="ps", name=f"ps1_{ot}_{b2}")
                for b2 in range(2)
            ]
            if ot == OTS[0]:
                # absorb the SP-queue wait for bf1[0] so the first matmul
                # carries only the PSUM-free (ACT copy) wait
                nc.tensor.ldweights(bf_sb[1][0][:, 0:128])
            if ot != OTS[-1]:
                for kt in range(KT):
                    lhsT = ct_sb[kt][:, ot * 128 : (ot + 1) * 128]
                    for b2 in range(2):
                        nc.tensor.matmul(
                            pair[b2][:],
                            lhsT=lhsT,
                            rhs=bf_sb[1][kt][:, b2 * 512 : (b2 + 1) * 512],
                            start=(kt == 0),
                            stop=(kt == KT - 1),
                        )
                drain_pair(pair, ot, 1, f"h1_{ot}")
            else:
                # Last pass: run the two 32-matmul chains back to back so the
                # first chain's drain + output DMA overlap the second chain's
                # ~7us of matmuls; only one 128KB chunk remains after the
                # final matmul.
                for b2 in range(2):
                    for kt in range(KT):
                        nc.tensor.matmul(
                            pair[b2][:],
                            lhsT=ct_sb[kt][:, ot * 128 : (ot + 1) * 128],
                            rhs=bf_sb[1][kt][:, b2 * 512 : (b2 + 1) * 512],
                            start=(kt == 0),
                            stop=(kt == KT - 1),
                        )
                    c0 = BH + b2 * 512
                    if b2 == 0:
                        st = stp.tile(
                            [128, 512], FP16, tag="st", name=f"st_h1_{ot}_{b2}"
                        )
                        nc.scalar.copy(st[:], pair[0][:])
                        g = len(drained)
                        nc.gpsimd.tensor_copy(scratch[:, g : g + 1], st[0:1, 0:1])
                        nc.gpsimd.dma_start(
                            out=out_d[ot * 128 : (ot + 1) * 128, c0 : c0 + 512],
                            in_=st[:],
                        )
                    else:
                        # Very last chunk: ACT and DVE copy one half each
                        # into separate staging tiles in parallel, halving
                        # the copy latency on the exposed tail path.
                        g = len(drained)
                        sthalves = []
                        for half, ecopy in ((0, nc.scalar.copy),
                                            (1, nc.vector.tensor_copy)):
                            sth = stp.tile(
                                [128, 256], FP16, tag="st",
                                name=f"st_h1_{ot}_{b2}_{half}",
                            )
                            if half == 1:
                                # The framework serializes the two PSUM-bank
                                # readers; a 1-elem DVE touch absorbs the
                                # ACT-copy dep so the real copy carries only
                                # the PE chain-stop wait.
                                nc.vector.tensor_copy(
                                    scratch[:, 20:21], sthalves[0][0:1, 0:1]
                                )
                            sthalves.append(sth)
                            ecopy(
                                sth[:],
                                pair[1][:, half * 256 : (half + 1) * 256],
                            )
                            nc.gpsimd.tensor_copy(
                                scratch[:, g + half : g + half + 1],
                                sth[0:1, 0:1],
                            )
                            nc.gpsimd.dma_start(
                                out=out_d[
                                    ot * 128 : (ot + 1) * 128,
                                    c0 + half * 256 : c0 + (half + 1) * 256,
                                ],
                                in_=sth[:],
                            )
                    drained.append(None)

    _cache["nc"] = nc
    return nc


def _make_in_maps(x, coefficients):
    L = _legendre_basis_np(np.asarray(x, dtype=np.float32))  # [8192, 3, 16]
    CT = np.ascontiguousarray(np.asarray(coefficients, dtype=np.float32).T)
    # Bfull[:, 0] == 1 exactly, so C'[0,:] = 1 yields
    # Bfull @ C'^T == 1 + Bfull @ C^T - C[:,0] (the reference expression).
    CT[0, :] = 1.0
    CT16 = CT.astype(np.float16)

    in_maps = []
    for c in range(NB * NO):
        bs, osh = c % NB, c // NB
        Lb = L[bs * BC : (bs + 1) * BC]  # [BC, 3, 16]
        # BfullT[(i,j,k), b] in fp16, built from fp32 factors
        bfull = np.einsum("bi,bj,bk->ijkb", Lb[:, 0], Lb[:, 1], Lb[:, 2])
        bfull = bfull.reshape(NFEAT, BC).astype(np.float16)
        in_maps.append(
            {
                "bf0": np.ascontiguousarray(bfull[:, :BH]),
                "bf1": np.ascontiguousarray(bfull[:, BH:]),
                "ct": np.ascontiguousarray(CT16[:, osh * OC : (osh + 1) * OC]),
            }
        )
    return in_maps


def _assemble(results):
    out = np.empty((BATCH, OUT), dtype=np.float32)
    for c in range(NB * NO):
        bs, osh = c % NB, c // NB
        out[bs * BC : (bs + 1) * BC, osh * OC : (osh + 1) * OC] = (
            results[c]["outT"].astype(np.float32).T
        )
    return out


def _run(x, coefficients, trace=False, **kwargs):
    nc = _build_program()
    in_maps = _make_in_maps(x, coefficients)
    res = run_bass_kernel_spmd(
        nc, in_maps, list(range(NB * NO)), trace=trace, **kwargs
    )
    return _assemble(res.results), res


def kernel(x, coefficients):
    out, _ = _run(x, coefficients)
    return out
